# revision 106
# baseline (speedup 1.0000x reference)
"""Causal-self-attention (non-causal SDPA + RoPE) Bass kernel for 8 Trainium2 cores.

Sharding: head-parallel. 16 heads / 8 cores = 2 heads per core, all 4 batches.
Each core computes QKV projections for its 2 heads (tensor-parallel split of
Wqkv rows), RoPE, full attention for its 8 (batch, head) units, and a partial
output projection against its 128-column slice of Wout. The 8 partial outputs
(bf16) are summed on the host (the all-reduce of the tensor-parallel out-proj).

Key structure (vs the straightforward version):
  - PV matmul is transposed: stationary = exp-tile [s,128t] chunk, moving =
    V [s,64d] -> psum attn^T [t, d] at 64 rows/matmul (half the PE rows of
    moving-E PV). Softmax denominators come from 1-row ones-moving matmuls
    into the same-partition psum; normalization is then a per-partition
    tensor_scalar_mul on DVE (no broadcast matmuls).
  - attn^T is normalized to bf16, PE-transposed back to [d, t] for the
    out-projection (moving = attnS bf16).
  - Emission is a per-s-tile software pipeline: window(st) = scores(st),
    exp(st) on ScalarE, PV(st-1), plus a PE-cycle-weighted slice of filler
    (next batch's QKV proj / previous tcg's outproj) so the PE never idles
    and stays at max p-state.
  - RoPE multiplies/adds run on the Pool (GPSIMD) engine; DVE keeps the
    PSUM evictions. Output DMA is bf16.
"""

import numpy as np

EMBED = 1024
NUM_HEADS = 16
HEAD_DIM = 64
T = 2048
B = 4
NCORES = 8
M = T * B  # 8192
ROPE_BASE = 10000.0


def _build_program():
    import concourse.bass as bass  # noqa: F401
    import concourse.mybir as mybir
    import concourse.tile as tile
    from concourse import bacc

    dt = mybir.dt
    F32, F32R, BF16 = dt.float32, dt.float32r, dt.bfloat16
    AF = mybir.ActivationFunctionType

    nc = bacc.Bacc("TRN2", target_bir_lowering=False, debug=False,
                   num_devices=NCORES)

    xt = nc.dram_tensor("xt", [EMBED, M], BF16, kind="ExternalInput")
    wq = nc.dram_tensor("wq", [EMBED, 128], BF16, kind="ExternalInput")
    wk = nc.dram_tensor("wk", [EMBED, 128], BF16, kind="ExternalInput")
    wv = nc.dram_tensor("wv", [EMBED, 128], BF16, kind="ExternalInput")
    wo = nc.dram_tensor("wo", [128, EMBED], BF16, kind="ExternalInput")
    cosd = nc.dram_tensor("cosd", [128, T], F32, kind="ExternalInput")
    sind = nc.dram_tensor("sind", [128, T], F32, kind="ExternalInput")
    identd = nc.dram_tensor("identd", [128, 128], BF16, kind="ExternalInput")
    permd = nc.dram_tensor("permd", [128, 128], BF16, kind="ExternalInput")
    out = nc.dram_tensor("out", [EMBED, M], BF16, kind="ExternalOutput")
    import os
    DBG = bool(os.environ.get("BASS_KERNEL_DEBUG"))
    if DBG:
        dbg_qt = nc.dram_tensor("dbg_qt", [128, M], F32, kind="ExternalOutput")
        dbg_kt = nc.dram_tensor("dbg_kt", [128, M], F32, kind="ExternalOutput")
        dbg_v0 = nc.dram_tensor("dbg_v0", [128, 2048], BF16, kind="ExternalOutput")
        dbg_as = nc.dram_tensor("dbg_as", [128, 512], BF16, kind="ExternalOutput")
        dbg_pv = nc.dram_tensor("dbg_pv", [128, 512], F32, kind="ExternalOutput")
        dbg_den = nc.dram_tensor("dbg_den", [128, 8], F32, kind="ExternalOutput")

    ST = 16            # s-tiles per batch (2048/128)

    with tile.TileContext(nc) as tc:
        with (
            tc.tile_pool(name="const", bufs=1) as cpool,
            tc.tile_pool(name="xt", bufs=3) as xpool,
            tc.tile_pool(name="big", bufs=1) as big,
            tc.tile_pool(name="rt", bufs=3) as rtpool,
            tc.tile_pool(name="et", bufs=10) as epool,
            tc.tile_pool(name="asr", bufs=8) as asrpool,
            tc.tile_pool(name="asS", bufs=16) as aspool,
            tc.tile_pool(name="rc", bufs=2) as rcpool,
            tc.tile_pool(name="ob", bufs=3) as opool,
            tc.tile_pool(name="ps", bufs=2, space="PSUM") as ps,
            tc.tile_pool(name="pv", bufs=1, space="PSUM") as pvpool,
            tc.tile_pool(name="pm", bufs=1, space="PSUM") as pmpool,
            tc.tile_pool(name="pp", bufs=2, space="PSUM") as pp,
        ):
            # ---- constants ----
            wq_sb = cpool.tile([128, 1024], BF16, tag="wq")
            wk_sb = cpool.tile([128, 1024], BF16, tag="wk")
            wv_sb = cpool.tile([128, 1024], BF16, tag="wv")
            cos_sb = cpool.tile([128, T], F32, tag="cos")
            sin_sb = cpool.tile([128, T], F32, tag="sin")
            identb = cpool.tile([128, 128], BF16, tag="identb")
            perm_sb = cpool.tile([128, 128], BF16, tag="perm")
            ones_bf = cpool.tile([128, 1], BF16, tag="ones_bf")
            zeros_sb = cpool.tile([128, 512], BF16, tag="zeros")
            wo_sb = cpool.tile([128, 1024], BF16, tag="wo")

            def _w_load(w_sb, w_d):
                nc.sync.dma_start(
                    w_sb[:].rearrange("p (e d) -> p e d", e=8),
                    w_d[:].rearrange("(e p) d -> p e d", e=8))

            warm = cpool.tile([1, 64], F32, tag="warm")
            qt_sb = big.tile([128, M], F32R, tag="qt")
            kt_sb = big.tile([128, M], F32R, tag="kt")
            # V per batch: [s, st*128 + head*64 + d] bf16
            v_sb = [big.tile([128, ST * 128], BF16, tag=f"v{b}", name=f"v_sb{b}")
                    for b in range(B)]

            # psum bank maps. HW rule: any start=True matmul zeroes its
            # whole PSUM bank, so each accumulating bank is zeroed once per
            # tcg by a full-region matmul against zeros, and every real
            # accumulation runs start=False.
            # pv: one bank, 8 accumulation regions of [128t, 64d] per tcg
            pv = pvpool.tile([128, 512], F32, tag="pv")
            # pm: one bank, denominators only [128, 8]
            pm = pmpool.tile([128, 512], F32, tag="pm")
            pm_den = pm[:, 0:8]

            def load_x_half(h, mc):
                """One DMA for all 8 e-chunks of X^T (512 m-cols)."""
                c0 = h * 1024 + mc * 512
                xts = xpool.tile([128, 8 * 512], BF16, tag="xt",
                                 name=f"xt{h}_{mc}")
                nc.sync.dma_start(
                    xts[:].rearrange("p (e m) -> p e m", e=8),
                    xt[:, c0:c0 + 512].rearrange("(e p) m -> p e m", e=8))
                return xts

            def rope(p, dst, col0, rush=False):
                """dst = cos*p + sin_eff*shift32(p), all [128, 512].

                The rotate-half partition shuffle is a PE matmul against a
                constant permutation matrix (sign lives in the sin table).
                Steady state: DVE copy + sin-mul, Pool cos-mul + add.
                rush=True (latency-critical opening): muls on DVE, Pool only
                adds, halving the serial chain to first-ready qt/kt."""
                pr = rtpool.tile([128, 512], BF16, tag="proj_sb")
                nc.vector.tensor_copy(pr[:], p if isinstance(p, type(pr[:])) else p[:])
                prs = pp.tile([128, 512], F32, tag="pp", name=f"prs_{col0}")
                nc.tensor.matmul(prs[:], perm_sb[:], pr[:],
                                 start=True, stop=True)
                tc0 = col0 % T
                u = rtpool.tile([128, 512], F32, tag="ropetmp")
                mul_eng = nc.vector if rush else nc.gpsimd
                mul_eng.tensor_mul(u[:], pr[:], cos_sb[:, tc0:tc0 + 512])
                t2 = rtpool.tile([128, 512], F32, tag="ropetmp2")
                nc.vector.tensor_mul(t2[:], prs[:], sin_sb[:, tc0:tc0 + 512])
                nc.gpsimd.tensor_add(dst, u[:], t2[:])

            # ---------- filler thunk machinery ----------
            # Each thunk: (pe_cycles, fn). fn() emits instructions.
            # Two priorities: "fast" (attnS transposes + outproj — must drain
            # within the next tcg so their tile slots recycle) and "slow"
            # (next batch's projections — drain across the whole batch).
            fast_q, slow_q, defer_q = [], [], []
            fast_pos, slow_pos, defer_pos = [0], [0], [0]
            fill_done = [0.0]
            fill_target = [0.0]

            def _pop_one():
                if fast_pos[0] < len(fast_q):
                    w, fn = fast_q[fast_pos[0]]
                    fast_pos[0] += 1
                elif slow_pos[0] < len(slow_q):
                    w, fn = slow_q[slow_pos[0]]
                    slow_pos[0] += 1
                elif defer_pos[0] < len(defer_q):
                    w, fn = defer_q[defer_pos[0]]
                    defer_pos[0] += 1
                else:
                    return False
                fn()
                fill_done[0] += w
                return True

            def emit_filler(budget_cycles):
                fill_target[0] += budget_cycles
                while fill_done[0] < fill_target[0]:
                    if not _pop_one():
                        break

            def drain_filler():
                while _pop_one():
                    pass

            # ---------- projection chunk (as thunks) ----------
            def proj_thunks(h, mc, xts, wide=False):
                """Thunks for one 512-m-col chunk: Q, K (f32r + rope) and V
                (bf16, transposed into v_sb). wide=True (opening): Q and K
                psums borrow the two banks of an idle sab-pool tile so four
                projection groups can be in flight at once."""
                b = h // 2
                col0 = h * 1024 + mc * 512
                st0 = (col0 % T) // 128
                state = {}

                def qk(which, w_sb, dst, lo, hi):
                    def fn():
                        if lo == 0:
                            if wide:
                                if 'qk' not in state:
                                    state['qk'] = ps.tile(
                                        [128, 1024], F32, tag="sab",
                                        name=f"qkps_{h}_{mc}")
                                half = 0 if which == 'q' else 512
                                state[which] = state['qk'][:, half:half + 512]
                            else:
                                state[which] = pp.tile(
                                    [128, 512], F32,
                                    tag="pp", name=f"pp_{which}_{h}_{mc}")
                        p = state[which]
                        for e in range(lo, hi):
                            nc.tensor.matmul(
                                p if wide else p[:],
                                w_sb[:, e * 128:(e + 1) * 128],
                                xts[:, e * 512:(e + 1) * 512],
                                start=(e == 0), stop=(e == 7))
                        if hi == 8:
                            rope(p if wide else p[:],
                                 dst[:, col0:col0 + 512], col0, rush=(h < 2))
                    return fn

                def vmm(mqs):
                    def fn():
                        if mqs[0] == 0:
                            state['v'] = pp.tile([128, 512], F32, tag="pp",
                                                 name=f"pp_v_{h}_{mc}")
                            nc.tensor.matmul(state['v'][:], identb[:],
                                             zeros_sb[:], start=True, stop=True)
                        p = state['v']
                        for mq in mqs:
                            # transposed V proj: out [m(=s) 128, d 128]
                            for e in range(8):
                                nc.tensor.matmul(
                                    p[:, mq * 128:(mq + 1) * 128],
                                    xts[:, e * 512 + mq * 128:e * 512 + mq * 128 + 128],
                                    wv_sb[:, e * 128:(e + 1) * 128],
                                    start=False, stop=(e == 7),
                                    skip_group_check=True)
                            nc.vector.tensor_copy(
                                v_sb[b][:, (st0 + mq) * 128:(st0 + mq + 1) * 128],
                                p[:, mq * 128:(mq + 1) * 128])
                    return fn

                th = []
                for lo in range(0, 8, 4):
                    th.append((512 * 4, qk('q', wq_sb, qt_sb, lo, lo + 4)))
                for lo in range(0, 8, 4):
                    th.append((512 * 4, qk('k', wk_sb, kt_sb, lo, lo + 4)))
                th.append((512 + 2048, vmm((0, 1))))
                th.append((2048, vmm((2, 3))))
                return th

            # ---------- outproj (as thunks) ----------
            def outproj_thunks(b, tcg, attnS, last=False):
                state = {}

                def ft_fn(ft):
                    def fn():
                        if ft % 4 == 0:
                            state['o'] = opool.tile([128, 4 * 512], BF16,
                                                    tag="ob",
                                                    name=f"osb_{b}_{tcg}_{ft}")
                        po = pp.tile([128, 512], F32, tag="pp",
                                     name=f"po_{b}_{tcg}_{ft}")
                        nc.tensor.matmul(po[:],
                                         wo_sb[:, ft * 128:(ft + 1) * 128],
                                         attnS[:],
                                         start=True, stop=True)
                        o_sb = state['o']
                        sl = o_sb[:, (ft % 4) * 512:(ft % 4 + 1) * 512]
                        if last and ft >= 4:
                            nc.scalar.activation(sl, po[:], AF.Copy)
                        else:
                            nc.vector.tensor_copy(sl, po[:])
                        if ft % 4 == 3:
                            # one batched store per half-tcg (4 f-blocks)
                            c0 = b * T + tcg * 512
                            f0 = ft - 3
                            nc.sync.dma_start(
                                out[f0 * 128:(f0 + 4) * 128,
                                    c0:c0 + 512].rearrange(
                                    "(f p) m -> p f m", f=4),
                                o_sb[:].rearrange("p (f m) -> p f m", f=4))
                    return fn
                return [(512, ft_fn(ft)) for ft in range(8)]

            # ---------- attention ----------
            def scores_exp(b, tcg, st):
                """scores(st) into a rotating sab buffer + exp on ScalarE.
                Returns the e_t tile."""
                c0 = b * T + tcg * 512
                s0 = b * T + st * 128
                sab = ps.tile([128, 1024], F32, tag="sab")
                nc.tensor.matmul(sab[:, 0:512],
                                 kt_sb[0:64, s0:s0 + 128],
                                 qt_sb[0:64, c0:c0 + 512],
                                 start=True, stop=True)
                nc.tensor.matmul(sab[:, 512:1024],
                                 kt_sb[64:128, s0:s0 + 128],
                                 qt_sb[64:128, c0:c0 + 512],
                                 start=True, stop=True)
                e_t = epool.tile([128, 1024], BF16, tag="et")
                nc.scalar.activation(e_t[:], sab[:], AF.Exp, scale=0.125)
                return e_t

            def zero_attn_banks():
                nc.tensor.matmul(pv[:], identb[:], zeros_sb[:],
                                 start=True, stop=True)
                # covers the boundary-transpose scratch too: the overlap
                # orders this zero behind the transpose evictions
                nc.tensor.matmul(pm[:, 0:64], identb[:], zeros_sb[:, 0:64],
                                 start=True, stop=True)

            def pv_mms(b, st, e_t):
                """Transposed PV + denominator mms for s-tile st."""
                for tcq in range(4):
                    for hh in range(2):
                        g = tcq * 2 + hh
                        lhs = e_t[:, hh * 512 + tcq * 128:hh * 512 + (tcq + 1) * 128]
                        nc.tensor.matmul(
                            pv[:, g * 64:(g + 1) * 64], lhs,
                            v_sb[b][:, st * 128 + hh * 64:st * 128 + hh * 64 + 64],
                            start=False, stop=(st == ST - 1),
                            skip_group_check=True)
                        nc.tensor.matmul(
                            pm_den[:, g:g + 1], lhs, ones_bf[:],
                            start=False, stop=(st == ST - 1),
                            skip_group_check=True)

            def finish_tcg(b, tcg):
                """Normalize attn^T, transpose to [d, t], return attnS tile."""
                # Snapshot psum to SBUF with two quick copies so the next
                # tcg's bank zero-matmuls wait only on these, not on the
                # whole normalize chain.
                pvs = rcpool.tile([128, 512], F32, tag="pvsnap")
                nc.vector.tensor_copy(pvs[:], pv[:])
                dens = rcpool.tile([128, 8], F32, tag="densnap")
                nc.vector.tensor_copy(dens[:], pm_den)
                if DBG and (b, tcg) == (0, 0):
                    nc.sync.dma_start(dbg_pv[:], pvs[:])
                    nc.sync.dma_start(dbg_den[:], dens[:])
                rec = rcpool.tile([128, 8], F32, tag="rec")
                attnS = aspool.tile([128, 512], BF16, tag="attnS",
                                    name=f"attnS_{b}_{tcg}")
                asr = {}
                with nc.allow_low_precision(reason="softmax denom recip"):
                    nc.vector.reciprocal(rec[:], dens[:])
                for tcq in range(4):
                    asr[tcq] = asrpool.tile([128, 128], BF16, tag="asr",
                                            name=f"asr_{b}_{tcg}_{tcq}")
                    for hh in range(2):
                        g = tcq * 2 + hh
                        nc.vector.tensor_scalar_mul(
                            asr[tcq][:, hh * 64:(hh + 1) * 64],
                            pvs[:, g * 64:(g + 1) * 64],
                            rec[:, g:g + 1])

                # Boundary transposes: PE transpose into the pm bank's
                # cols 0:64 (bf16 view). Writing over the denominator region
                # gives the exact dependency chain: transpose waits the dens
                # snapshot; the next tcg's pm zero-matmul waits the evicts.
                trb = pm[:, 0:64].bitcast(BF16)
                for tcq in range(4):
                    nc.tensor.transpose(trb, asr[tcq][:], identb[:])
                    nc.vector.tensor_copy(
                        attnS[:, tcq * 128:(tcq + 1) * 128], trb)
                return attnS, []

            # ---------- emission ----------
            nc.scalar.activation(warm[:], identb[0:1, :].bitcast(F32),
                                 AF.Exp, scale=0.0)
            nc.vector.memset(ones_bf[:], 1.0)
            nc.vector.memset(zeros_sb[:], 0.0)

            # batch-0 projection runs in the open (PE otherwise idle).
            # DMA order follows the opening critical chain: wq -> chunk0 ->
            # cos -> wk -> sin -> chunk1 -> wv -> ... so the first matmuls
            # and first ropes are never DMA-starved.
            chunk_order = [(h, mc) for h in range(8) for mc in range(2)]
            xts_pending = {}
            _w_load(wq_sb, wq)
            nc.sync.dma_start(identb[:], identd[:])
            nc.sync.dma_start(perm_sb[:], permd[:])
            xts_pending[chunk_order[0]] = load_x_half(*chunk_order[0])
            nc.sync.dma_start(cos_sb[:], cosd[:])
            _w_load(wk_sb, wk)
            nc.sync.dma_start(sin_sb[:], sind[:])
            xts_pending[chunk_order[1]] = load_x_half(*chunk_order[1])
            _w_load(wv_sb, wv)
            nc.sync.dma_start(wo_sb[:], wo[:])
            for (h, mc) in chunk_order[2:4]:
                xts_pending[(h, mc)] = load_x_half(h, mc)
            # chunks (0,0)/(0,1) inline; (1,0)/(1,1) lead the slow queue so
            # tcg-0 windows 0-7 can start while they project
            for ci, (h, mc) in enumerate(chunk_order[:2]):
                for _, fn in proj_thunks(h, mc, xts_pending.pop((h, mc)),
                                         wide=True):
                    fn()
            for ci in (2, 3):
                h, mc = chunk_order[ci]
                nh, nmc = chunk_order[ci + 2]
                xts_pending[(nh, nmc)] = load_x_half(nh, nmc)
                slow_q.extend(proj_thunks(h, mc, xts_pending.pop((h, mc)),
                                          wide=True))

            # enqueue helper: chunk DMAs issued two chunks ahead
            next_dma = [6]

            def enqueue_proj(ci):
                h, mc = chunk_order[ci]
                if (h, mc) not in xts_pending:
                    xts_pending[(h, mc)] = load_x_half(h, mc)
                xts = xts_pending.pop((h, mc))
                th = proj_thunks(h, mc, xts)

                def prefetch():
                    if next_dma[0] < len(chunk_order):
                        nh, nmc = chunk_order[next_dma[0]]
                        xts_pending[(nh, nmc)] = load_x_half(nh, nmc)
                        next_dma[0] += 1
                w0, f0 = th[0]

                def first():
                    prefetch()
                    f0()
                slow_q.append((w0, first))
                slow_q.extend(th[1:])

            # Per-window filler budget (PE cycles). Chosen so the slow queue
            # drains each batch's projections within the preceding batch's
            # attention phase even after the fast queue takes its share.
            FILL_W = 750.0

            prev = None          # (b, tcg, attnS)
            attnS_dbg = {}
            for b in range(B):
                if b + 1 < B:
                    for ci in range(4 * (b + 1), 4 * (b + 2)):
                        enqueue_proj(ci)
                else:
                    # final phase: fold the deferred outproj backlog into the
                    # normal drain order so it spreads across these windows
                    slow_q.extend(defer_q[defer_pos[0]:])
                    defer_pos[0] = len(defer_q)
                for tcg in range(4):
                    zero_attn_banks()
                    pend = None
                    for st in range(ST):
                        e_t = scores_exp(b, tcg, st)
                        if pend is not None:
                            pv_mms(b, pend[0], pend[1])
                        pend = (st, e_t)
                        boost = 2600 if (b == 0 and tcg == 0) else 0
                        emit_filler(FILL_W + boost + (520 if st == 0 else 0))
                    pv_mms(b, pend[0], pend[1])
                    attnS, tr_th = finish_tcg(b, tcg)
                    fast_q.extend(tr_th)
                    if prev is not None:
                        # split outproj: half drains next phase (keeps DVE
                        # evictions spread out), half defers to the final
                        # phase (keeps store-DMAs off the xt-load phases)
                        th = outproj_thunks(prev[0], prev[1], prev[2])
                        if (prev[1] % 2) == 1:
                            defer_q.extend(th)
                        else:
                            slow_q.extend(th)
                    prev = (b, tcg, attnS)
                    if (b, tcg) == (3, 0):
                        attnS_dbg[0] = attnS
            drain_filler()
            for _, fn in outproj_thunks(prev[0], prev[1], prev[2], last=True):
                fn()
            if DBG:
                nc.sync.dma_start(dbg_qt[:], qt_sb[:].bitcast(F32))
                nc.sync.dma_start(dbg_kt[:], kt_sb[:].bitcast(F32))
                nc.sync.dma_start(dbg_v0[:], v_sb[3][:])
                nc.sync.dma_start(dbg_as[:], attnS_dbg[0][:])


    nc.compile()
    return nc


def _host_prep(query, Wqkv, Wout):
    import ml_dtypes

    q32 = np.asarray(query, dtype=np.float32)
    # [T, B, E] -> [E, B, T] -> [E, B*T]  (column = b*T + t)
    xt = np.ascontiguousarray(q32.transpose(2, 1, 0).reshape(EMBED, M))

    # rope tables, fp16-rounded like the reference
    theta = np.power(ROPE_BASE,
                     -np.arange(0, HEAD_DIM, 2, dtype=np.float32) / HEAD_DIM)
    m_th = np.arange(T, dtype=np.float32)[:, None] * theta[None, :]
    m_th = np.concatenate([m_th, m_th], axis=-1)          # [T, 64]
    cos = np.cos(m_th).astype(np.float16).astype(np.float32)
    sin = np.sin(m_th).astype(np.float16).astype(np.float32)
    cosT = cos.T                                          # [64, T]
    sin_eff = sin.T.copy()
    sin_eff[0:32] = -sin_eff[0:32]
    cos128 = np.ascontiguousarray(np.concatenate([cosT, cosT], axis=0))
    sin128 = np.ascontiguousarray(np.concatenate([sin_eff, sin_eff], axis=0))

    # rotate-half permutation as a stationary matrix: prs = perm^T @ pr,
    # prs[i] = pr[perm(i)] with perm swapping 32-blocks within each 64-half
    perm = np.zeros((128, 128), dtype=np.float32)
    for i in range(128):
        j = (i // 64) * 64 + (i + 32) % 64
        perm[j, i] = 1.0
    perm = perm.astype(ml_dtypes.bfloat16)

    W = np.asarray(Wqkv, dtype=np.float32)
    Wo = np.asarray(Wout, dtype=np.float32)
    in_maps = []
    for c in range(NCORES):
        sl = slice(c * 128, (c + 1) * 128)
        in_maps.append({
            "xt": xt.astype(ml_dtypes.bfloat16),
            "wq": np.ascontiguousarray(W[sl, :].T).astype(ml_dtypes.bfloat16),
            "wk": np.ascontiguousarray(W[EMBED:][sl, :].T).astype(
                ml_dtypes.bfloat16),
            "wv": np.ascontiguousarray(W[2 * EMBED:][sl, :].T).astype(
                ml_dtypes.bfloat16),
            "wo": np.ascontiguousarray(Wo[:, sl].T).astype(ml_dtypes.bfloat16),
            "cosd": cos128,
            "sind": sin128,
            "identd": np.eye(128, dtype=np.float32).astype(ml_dtypes.bfloat16),
            "permd": perm,
        })
    return in_maps


def kernel(query, Wqkv, Wout):
    from concourse.bass_utils import run_bass_kernel_spmd

    nc = _build_program()
    in_maps = _host_prep(query, Wqkv, Wout)
    res = run_bass_kernel_spmd(nc, in_maps, core_ids=list(range(NCORES)))
    acc = np.zeros((EMBED, M), dtype=np.float32)
    for r in res.results:
        acc += np.asarray(r["out"], dtype=np.float32)
    # out^T [E, b*T+t] -> [B, T, E] -> [T, B, E]
    full = acc.T.reshape(B, T, EMBED).transpose(1, 0, 2)
    return np.ascontiguousarray(full)
